# revision 17
# baseline (speedup 1.0000x reference)
"""Trainium2 Bass kernel for the SuperGlue-style cross-attention block.

Math (validated in fp32 to ~2e-7 vs the jax reference):
  x = descs.T  [C=256, N=8192]
  q/k/v head-permuted so each head owns a contiguous 64-row block.
  Per head: S^T[m,n] = k_h[:,m].q_h[:,n];  P^T = exp(S^T/8 + r_h[m])  (no max
  subtraction needed: |S/8| < ~1);  r_h folds the bq bias exactly.
  PV with a ones-column appended to v^T gives unnormalized msg + denominator
  in one accumulated matmul; normalization happens on the PV output.
  Wm is folded into W1 host-side (W1b = W1[:,256:]@Wm).  bk/bv/bm/b1 are
  provably no-ops (softmax shift invariance / InstanceNorm mean removal);
  b2 and the residual are applied host-side.
  InstanceNorm statistics are combined across the 8 query-shard cores with a
  tiny [128,4,2] AllReduce.

Sharding: query axis N split 8 ways (sequence parallel); k/v computed from the
replicated full x on every core; params replicated.
"""

import numpy as np
import ml_dtypes

import concourse.bass as bass
import concourse.tile as tile
from concourse import bacc
import concourse.mybir as mybir
from concourse.bass_utils import run_bass_kernel_spmd

NCORES = 8
C = 256
N = 8192
H = 4
DIM = 64
NL = N // NCORES  # 1024 queries per core
EPS = 1e-5

F32 = mybir.dt.float32
BF16 = mybir.dt.bfloat16
AF = mybir.ActivationFunctionType
ALU = mybir.AluOpType
BF16_NP = ml_dtypes.bfloat16


def build_kernel():
    nc = bacc.Bacc("TRN2", target_bir_lowering=False, debug=False,
                   num_devices=NCORES)

    # ---- DRAM I/O (all weight marshalling is done host-side) ----
    xl_d = nc.dram_tensor("xl", [2, 128, NL], BF16, kind="ExternalInput")
    xf_d = nc.dram_tensor("xf", [2, 128, N], BF16, kind="ExternalInput")
    wq_d = nc.dram_tensor("wq", [2, 128, C], BF16, kind="ExternalInput")
    wk_d = nc.dram_tensor("wk", [2, 128, C], BF16, kind="ExternalInput")
    wva_d = nc.dram_tensor("wva", [2, 128, 260], BF16, kind="ExternalInput")
    w1a_d = nc.dram_tensor("w1a", [2, 128, 2 * C], BF16, kind="ExternalInput")
    w1b_d = nc.dram_tensor("w1b", [2, 128, 2 * C], BF16, kind="ExternalInput")
    w2_d = nc.dram_tensor("w2", [4, 128, C], BF16, kind="ExternalInput")
    delta_d = nc.dram_tensor("delta", [2, 128, NL], F32, kind="ExternalOutput")

    MT = N // 128  # 64 key tiles

    with tile.TileContext(nc) as tc:
        with tc.tile_pool(name="persist", bufs=1) as persist, \
             tc.tile_pool(name="dram", bufs=1, space="DRAM") as dram:

            # ---- persistent SBUF tensors ----
            xl_sb = persist.tile([128, 2, NL], BF16)
            xf_sb = persist.tile([128, 2, N], BF16)
            wq_sb = persist.tile([128, 2, C], BF16)
            wk_sb = persist.tile([128, 2, C], BF16)
            wva_sb = persist.tile([128, 2, 260], BF16)
            w1a_sb = persist.tile([128, 2, 2 * C], BF16)
            w1b_sb = persist.tile([128, 2, 2 * C], BF16)
            w2_sb = persist.tile([128, 4, C], BF16)
            q_sb = persist.tile([128, 2, NL], BF16)
            k_sb = persist.tile([128, 2, N], BF16)
            vt_sb = persist.tile([128, MT, H, 65], BF16)
            r_sb = persist.tile([128, MT, H], F32)
            msg_sb = persist.tile([128, 2, NL], BF16)
            y_sb = persist.tile([128, 4, NL], BF16)
            yn_sb = persist.tile([128, 4, NL], BF16)

            for ct in range(2):
                nc.sync.dma_start(out=xl_sb[:, ct, :], in_=xl_d[ct])
                nc.sync.dma_start(out=wq_sb[:, ct, :], in_=wq_d[ct])
                nc.sync.dma_start(out=wk_sb[:, ct, :], in_=wk_d[ct])
                nc.sync.dma_start(out=wva_sb[:, ct, :], in_=wva_d[ct])
            for ch in range(8):  # chunk-major so early k tiles unblock fast
                s = slice(ch * (N // 8), (ch + 1) * (N // 8))
                for ct in range(2):
                    nc.sync.dma_start(out=xf_sb[:, ct, s], in_=xf_d[ct, :, s])
            for ct in range(2):
                nc.sync.dma_start(out=w1a_sb[:, ct, :], in_=w1a_d[ct])
                nc.sync.dma_start(out=w1b_sb[:, ct, :], in_=w1b_d[ct])
            for ct in range(4):
                nc.sync.dma_start(out=w2_sb[:, ct, :], in_=w2_d[ct])

            # ones column of v~^T
            nc.vector.memset(vt_sb[:, :, :, 64:65], 1.0)

            # ---- PE warmup: ~7us of dummy back-to-back matmuls ----
            # HAM only unthrottles after a fully-busy 4096-cycle window and
            # keeps the warm state as long as idle gaps stay < ~3.4us; enter
            # every later phase warm.
            dum_sb = persist.tile([128, 512], BF16)
            nc.vector.memset(dum_sb[:, :], 0.0)
            with tc.tile_pool(name="wu", bufs=2, space="PSUM") as wu:
                for i in range(24):
                    ps = wu.tile([128, 512], F32, tag="wu", name=f"wu{i}")
                    nc.tensor.matmul(out=ps, lhsT=dum_sb[:, 0:128],
                                     rhs=dum_sb[:, :], start=True, stop=True)

            # ---- fused projections + attention ----
            # Software-pipelined per head: exp(mt) -> S^T(mt+1) -> PV(mt), so
            # in PE program order S^T(mt+1) precedes PV(mt) and ACT's wait on
            # "S^T(mt+1) done" (monotonic PE tick) never transitively waits
            # on PV.  st bufs=3 (6 banks) + one PV accumulator (2 banks).
            # Projection tiles share the st slots and fill PE spare capacity.
            with tc.tile_pool(name="st", bufs=2, space="PSUM") as st_pool, \
                 tc.tile_pool(name="pv", bufs=2, space="PSUM") as pv_pool, \
                 tc.tile_pool(name="pb", bufs=8) as pb_pool, \
                 tc.tile_pool(name="nrm", bufs=4) as nrm_pool:

                def q_proj(ot):
                    ps = st_pool.tile([128, NL], F32, tag="st", name=f"qp{ot}")
                    for i in range(2):
                        for ct in range(2):
                            nc.tensor.matmul(
                                out=ps[:, i * 512:(i + 1) * 512],
                                lhsT=wq_sb[:, ct, ot * 128:(ot + 1) * 128],
                                rhs=xl_sb[:, ct, i * 512:(i + 1) * 512],
                                start=(ct == 0), stop=(ct == 1))
                    nc.vector.tensor_copy(out=q_sb[:, ot, :], in_=ps)

                def k_proj(ot, i):
                    ps = st_pool.tile([128, 512], F32, tag="st",
                                      name=f"kp{ot}_{i}")
                    for ct in range(2):
                        nc.tensor.matmul(
                            out=ps,
                            lhsT=wk_sb[:, ct, ot * 128:(ot + 1) * 128],
                            rhs=xf_sb[:, ct, i * 512:(i + 1) * 512],
                            start=(ct == 0), stop=(ct == 1))
                    nc.vector.tensor_copy(
                        out=k_sb[:, ot, i * 512:(i + 1) * 512], in_=ps)

                def vt_proj(mt):
                    ps = st_pool.tile([128, 260], F32, tag="st",
                                      name=f"vp{mt}")
                    for ct in range(2):
                        nc.tensor.matmul(
                            out=ps,
                            lhsT=xf_sb[:, ct, mt * 128:(mt + 1) * 128],
                            rhs=wva_sb[:, ct, :],
                            start=(ct == 0), stop=(ct == 1))
                    nc.vector.tensor_copy(
                        out=vt_sb[:, mt, :, 0:64],
                        in_=ps[:, 0:256].rearrange("p (h d) -> p h d", h=H))
                    nc.vector.tensor_copy(out=r_sb[:, mt, :], in_=ps[:, 256:260])

                def norm_msg(h, pv):
                    pair, hh = h // 2, h % 2
                    rows = slice(hh * 64, (hh + 1) * 64)
                    rcp = nrm_pool.tile([1, NL], F32, tag="rcp",
                                        name=f"rcp{h}")
                    nc.vector.reciprocal(out=rcp, in_=pv[64:65, :])
                    rcp_d = dram.tile([1, NL], F32, tag="rcp_d",
                                      name=f"rcpd{h}")
                    nc.sync.dma_start(out=rcp_d, in_=rcp)
                    rb = nrm_pool.tile([64, NL], F32, tag="rb", name=f"rb{h}")
                    bc = bass.AP(tensor=rcp_d.tensor, offset=rcp_d.offset,
                                 ap=[[0, 64]] + list(rcp_d.ap[1:]))
                    nc.sync.dma_start(out=rb, in_=bc)
                    nc.vector.tensor_mul(
                        out=msg_sb[rows, pair, :], in0=pv[0:64, :], in1=rb)

                def st_mm(h, mt):
                    pair, hh = h // 2, h % 2
                    rows = slice(hh * 64, (hh + 1) * 64)
                    ps = st_pool.tile([128, NL], F32, tag="st",
                                      name=f"s{h}_{mt}")
                    for i in range(2):
                        nc.tensor.matmul(
                            out=ps[:, i * 512:(i + 1) * 512],
                            lhsT=k_sb[rows, pair, mt * 128:(mt + 1) * 128],
                            rhs=q_sb[rows, pair, i * 512:(i + 1) * 512],
                            start=True, stop=True)
                    return ps

                q_proj(0)
                for j in range(3):
                    k_proj(0, j)
                for mtp in range(3):
                    vt_proj(mtp)

                pvs = {}
                for h in range(H):
                    pair = h // 2
                    pv = pv_pool.tile([65, NL], F32, tag="pv", name=f"pv{h}")
                    pvs[h] = pv
                    sts = st_mm(h, 0)
                    for mt in range(MT):
                        h_ = 2 * pair + (h % 2)
                        pt = pb_pool.tile([128, NL], BF16, tag="pt",
                                          name=f"p{h}_{mt}")
                        nc.scalar.activation(
                            out=pt, in_=sts, func=AF.Exp,
                            bias=r_sb[:, mt, h:h + 1], scale=0.125)
                        # projection ride-alongs (keep them ahead of use)
                        if h == 0:
                            if mt % 4 == 0 and 3 + mt // 4 < 16:
                                k_proj(0, 3 + mt // 4)
                            if mt + 3 < MT:
                                vt_proj(mt + 3)
                        elif h == 1:
                            if mt == 0:
                                q_proj(1)
                            if mt % 4 == 2 and mt // 4 < 16:
                                k_proj(1, mt // 4)
                        if mt + 1 < MT:
                            sts = st_mm(h, mt + 1)
                        for i in range(2):
                            nc.tensor.matmul(
                                out=pv[:, i * 512:(i + 1) * 512],
                                lhsT=vt_sb[:, mt, h, :],
                                rhs=pt[:, i * 512:(i + 1) * 512],
                                start=(mt == 0), stop=(mt == MT - 1))
                    norm_msg(h, pv)

            # ---- y = W1a@x + W1b@msg ; per-core stats ----
            with tc.tile_pool(name="yp", bufs=4, space="PSUM") as yp, \
                 tc.tile_pool(name="stat", bufs=1) as stp:
                # bridge the attention->y boundary (norm chain ~8us)
                for i in range(40):
                    ps = yp.tile([128, 512], F32, tag="yp", name=f"wy{i}")
                    nc.tensor.matmul(out=ps, lhsT=dum_sb[:, 0:128],
                                     rhs=dum_sb[:, :], start=True, stop=True)
                st_t = stp.tile([128, 4, 2, 6], F32)
                mv_t = stp.tile([128, 4, 2], F32)
                ar_in = stp.tile([128, 4, 2], F32)
                tmp_m2 = stp.tile([128, 1], F32)
                for ot in range(4):
                    for i in range(2):
                        ps = yp.tile([128, 512], F32, tag="yp")
                        for ct in range(2):
                            nc.tensor.matmul(
                                out=ps,
                                lhsT=w1a_sb[:, ct, ot * 128:(ot + 1) * 128],
                                rhs=xl_sb[:, ct, i * 512:(i + 1) * 512],
                                start=(ct == 0), stop=False)
                        for ct in range(2):
                            nc.tensor.matmul(
                                out=ps,
                                lhsT=w1b_sb[:, ct, ot * 128:(ot + 1) * 128],
                                rhs=msg_sb[:, ct, i * 512:(i + 1) * 512],
                                start=False, stop=(ct == 1))
                        nc.vector.bn_stats(out=st_t[:, ot, i, :], in_=ps)
                        nc.scalar.copy(
                            out=y_sb[:, ot, i * 512:(i + 1) * 512], in_=ps)
                    nc.vector.bn_aggr(out=mv_t[:, ot, :], in_=st_t[:, ot, :, :])
                    # (mean, E[y^2]) for cross-core combination
                    nc.vector.tensor_mul(out=tmp_m2, in0=mv_t[:, ot, 0:1],
                                         in1=mv_t[:, ot, 0:1])
                    nc.vector.tensor_add(out=ar_in[:, ot, 1:2],
                                         in0=mv_t[:, ot, 1:2], in1=tmp_m2)
                    nc.vector.tensor_copy(out=ar_in[:, ot, 0:1],
                                          in_=mv_t[:, ot, 0:1])

                # ---- AllReduce of [128,4,2] stats ----
                ar_i_d = dram.tile([128, 4, 2], F32, tag="ar_i")
                ar_o_d = dram.tile([128, 4, 2], F32, tag="ar_o")
                nc.sync.dma_start(out=ar_i_d, in_=ar_in)
                nc.gpsimd.collective_compute(
                    "AllReduce", ALU.add,
                    replica_groups=[list(range(NCORES))],
                    ins=[ar_i_d.opt()], outs=[ar_o_d.opt()])
                ar_out = stp.tile([128, 4, 2], F32)
                nc.sync.dma_start(out=ar_out, in_=ar_o_d)

                # stats finalize: rs = 1/sqrt(var+eps), nm = -mu*rs
                mu_t = stp.tile([128, 4], F32)
                ey_t = stp.tile([128, 4], F32)
                rs_t = stp.tile([128, 4], F32)
                nm_t = stp.tile([128, 4], F32)
                nc.vector.tensor_scalar_mul(out=mu_t, in0=ar_out[:, :, 0],
                                            scalar1=1.0 / NCORES)
                nc.vector.tensor_scalar_mul(out=ey_t, in0=ar_out[:, :, 1],
                                            scalar1=1.0 / NCORES)
                nc.vector.tensor_mul(out=rs_t, in0=mu_t, in1=mu_t)
                nc.vector.tensor_sub(out=ey_t, in0=ey_t, in1=rs_t)  # var
                nc.vector.tensor_scalar_add(out=ey_t, in0=ey_t, scalar1=EPS)
                nc.scalar.activation(out=rs_t, in_=ey_t, func=AF.Sqrt)
                nc.vector.reciprocal(out=rs_t, in_=rs_t)
                nc.vector.tensor_mul(out=nm_t, in0=mu_t, in1=rs_t)
                nc.vector.tensor_scalar_mul(out=nm_t, in0=nm_t, scalar1=-1.0)

                # ---- yn = relu(y*rs + nm) ----
                for ot in range(4):
                    for i in range(2):
                        sl = slice(i * 512, (i + 1) * 512)
                        nc.vector.tensor_scalar(
                            out=yn_sb[:, ot, sl], in0=y_sb[:, ot, sl],
                            scalar1=rs_t[:, ot:ot + 1],
                            scalar2=nm_t[:, ot:ot + 1],
                            op0=ALU.mult, op1=ALU.add)
                        nc.vector.tensor_scalar_max(
                            out=yn_sb[:, ot, sl], in0=yn_sb[:, ot, sl],
                            scalar1=0.0)

                # ---- delta = W2 @ yn -> DRAM ----
                out_sb = stp.tile([128, 2, NL], F32)
                for ot in range(2):
                    for i in range(2):
                        ps = yp.tile([128, 512], F32, tag="yp")
                        for ct in range(4):
                            nc.tensor.matmul(
                                out=ps,
                                lhsT=w2_sb[:, ct, ot * 128:(ot + 1) * 128],
                                rhs=yn_sb[:, ct, i * 512:(i + 1) * 512],
                                start=(ct == 0), stop=(ct == 3))
                        sl = slice(i * 512, (i + 1) * 512)
                        if i == 0:
                            nc.scalar.copy(out=out_sb[:, ot, sl], in_=ps)
                        else:
                            nc.vector.tensor_copy(out=out_sb[:, ot, sl], in_=ps)
                        nc.sync.dma_start(
                            out=delta_d[ot, :, sl], in_=out_sb[:, ot, sl])

    nc.compile()
    return nc


_NC_CACHE = {}


def _get_nc():
    if "nc" not in _NC_CACHE:
        _NC_CACHE["nc"] = build_kernel()
    return _NC_CACHE["nc"]


def _prep(a, shape):
    return np.ascontiguousarray(
        np.asarray(a, np.float32).astype(BF16_NP).reshape(shape))


def kernel(descs, Wq, bq, Wk, bk, Wv, bv, Wm, bm, W1, b1, W2, b2):
    descs = np.asarray(descs, np.float32)
    Wq, bq = np.asarray(Wq, np.float32), np.asarray(bq, np.float32)
    Wk, bk = np.asarray(Wk, np.float32), np.asarray(bk, np.float32)
    Wv, bv = np.asarray(Wv, np.float32), np.asarray(bv, np.float32)
    Wm, bm = np.asarray(Wm, np.float32), np.asarray(bm, np.float32)
    W1, b1 = np.asarray(W1, np.float32), np.asarray(b1, np.float32)
    W2, b2 = np.asarray(W2, np.float32), np.asarray(b2, np.float32)

    x = descs[0].T  # [C, N] fp32
    perm = np.array([d * H + h for h in range(H) for d in range(DIM)])
    Wqp, Wkp, Wvp, bqp = Wq[perm], Wk[perm], Wv[perm], bq[perm]
    # wva: columns 0..255 = Wv'^T, 256+h = Wk'_h^T bq'_h / 8, pad to 260
    wva = np.zeros((C, 260), np.float32)
    wva[:, :C] = Wvp.T
    for h in range(H):
        sl = slice(h * DIM, (h + 1) * DIM)
        wva[:, C + h] = (Wkp[sl].T @ bqp[sl]) / np.sqrt(DIM)
    W1a = W1[:, :C]
    W1b = W1[:, C:] @ Wm[:, perm]

    xf = _prep(x, (2, 128, N))
    wq = _prep(Wqp.T, (2, 128, C))
    wk = _prep(Wkp.T, (2, 128, C))
    wva_b = _prep(wva, (2, 128, 260))
    w1a = _prep(W1a.T, (2, 128, 2 * C))
    w1b = _prep(W1b.T, (2, 128, 2 * C))
    w2 = _prep(W2.T, (4, 128, C))

    in_maps = []
    for i in range(NCORES):
        xl = _prep(x[:, i * NL:(i + 1) * NL], (2, 128, NL))
        in_maps.append(dict(xl=xl, xf=xf, wq=wq, wk=wk, wva=wva_b,
                            w1a=w1a, w1b=w1b, w2=w2))

    nc = _get_nc()
    _NC_CACHE["in_maps"] = in_maps
    res = run_bass_kernel_spmd(nc, in_maps, core_ids=list(range(NCORES)))
    delta = np.concatenate(
        [r["delta"].reshape(C, NL) for r in res.results], axis=1)  # [C, N]
    out = descs[0] + delta.T + b2[None, :]
    return out[None].astype(np.float32)


# revision 18
# speedup vs baseline: 1.2242x; 1.2242x over previous
"""Trainium2 Bass kernel for the SuperGlue-style cross-attention block.

Math (validated in fp32 to ~2e-7 vs the jax reference):
  x = descs.T  [C=256, N=8192]
  q/k/v head-permuted so each head owns a contiguous 64-row block.
  Per head: S^T[m,n] = k_h[:,m].q_h[:,n];  P^T = exp(S^T/8 + r_h[m])  (no max
  subtraction needed: |S/8| < ~1);  r_h folds the bq bias exactly.
  PV with a ones-column appended to v^T gives unnormalized msg + denominator
  in one accumulated matmul; normalization happens on the PV output.
  Wm is folded into W1 host-side (W1b = W1[:,256:]@Wm).  bk/bv/bm/b1 are
  provably no-ops (softmax shift invariance / InstanceNorm mean removal);
  b2 and the residual are applied host-side.
  InstanceNorm statistics are combined across the 8 query-shard cores with a
  tiny [128,4,2] AllReduce.

Sharding: query axis N split 8 ways (sequence parallel); k/v computed from the
replicated full x on every core; params replicated.
"""

import numpy as np
import ml_dtypes

import concourse.bass as bass
import concourse.tile as tile
from concourse import bacc
import concourse.mybir as mybir
from concourse.bass_utils import run_bass_kernel_spmd

NCORES = 8
C = 256
N = 8192
H = 4
DIM = 64
NL = N // NCORES  # 1024 queries per core
EPS = 1e-5

F32 = mybir.dt.float32
BF16 = mybir.dt.bfloat16
AF = mybir.ActivationFunctionType
ALU = mybir.AluOpType
BF16_NP = ml_dtypes.bfloat16


def build_kernel():
    nc = bacc.Bacc("TRN2", target_bir_lowering=False, debug=False,
                   num_devices=NCORES)

    # ---- DRAM I/O (all weight marshalling is done host-side) ----
    xl_d = nc.dram_tensor("xl", [2, 128, NL], BF16, kind="ExternalInput")
    xf_d = nc.dram_tensor("xf", [2, 128, N], BF16, kind="ExternalInput")
    wq_d = nc.dram_tensor("wq", [2, 128, C], BF16, kind="ExternalInput")
    wk_d = nc.dram_tensor("wk", [2, 128, C], BF16, kind="ExternalInput")
    wva_d = nc.dram_tensor("wva", [2, 128, 260], BF16, kind="ExternalInput")
    w1a_d = nc.dram_tensor("w1a", [2, 128, 2 * C], BF16, kind="ExternalInput")
    w1b_d = nc.dram_tensor("w1b", [2, 128, 2 * C], BF16, kind="ExternalInput")
    w2_d = nc.dram_tensor("w2", [4, 128, C], BF16, kind="ExternalInput")
    delta_d = nc.dram_tensor("delta", [2, 128, NL], F32, kind="ExternalOutput")

    MT = N // 128  # 64 key tiles

    with tile.TileContext(nc) as tc:
        with tc.tile_pool(name="persist", bufs=1) as persist, \
             tc.tile_pool(name="dram", bufs=1, space="DRAM") as dram:

            # ---- persistent SBUF tensors ----
            xl_sb = persist.tile([128, 2, NL], BF16)
            xf_sb = persist.tile([128, 2, N], BF16)
            wq_sb = persist.tile([128, 2, C], BF16)
            wk_sb = persist.tile([128, 2, C], BF16)
            wva_sb = persist.tile([128, 2, 260], BF16)
            w1a_sb = persist.tile([128, 2, 2 * C], BF16)
            w1b_sb = persist.tile([128, 2, 2 * C], BF16)
            w2_sb = persist.tile([128, 4, C], BF16)
            q_sb = persist.tile([128, 2, NL], BF16)
            k_sb = persist.tile([128, 2, N], BF16)
            vt_sb = persist.tile([128, MT, H, 65], BF16)
            r_sb = persist.tile([128, MT, H], F32)
            msg_sb = persist.tile([128, 2, NL], BF16)
            y_sb = persist.tile([128, 4, NL], BF16)
            yn_sb = persist.tile([128, 4, NL], BF16)

            for ct in range(2):
                nc.sync.dma_start(out=xl_sb[:, ct, :], in_=xl_d[ct])
                nc.sync.dma_start(out=wq_sb[:, ct, :], in_=wq_d[ct])
                nc.sync.dma_start(out=wk_sb[:, ct, :], in_=wk_d[ct])
                nc.sync.dma_start(out=wva_sb[:, ct, :], in_=wva_d[ct])
            for ch in range(8):  # chunk-major so early k tiles unblock fast
                s = slice(ch * (N // 8), (ch + 1) * (N // 8))
                for ct in range(2):
                    nc.sync.dma_start(out=xf_sb[:, ct, s], in_=xf_d[ct, :, s])
            for ct in range(2):
                nc.sync.dma_start(out=w1a_sb[:, ct, :], in_=w1a_d[ct])
                nc.sync.dma_start(out=w1b_sb[:, ct, :], in_=w1b_d[ct])
            for ct in range(4):
                nc.sync.dma_start(out=w2_sb[:, ct, :], in_=w2_d[ct])

            # ones column of v~^T
            nc.vector.memset(vt_sb[:, :, :, 64:65], 1.0)

            # ---- PE warmup: ~7us of dummy back-to-back matmuls ----
            # HAM only unthrottles after a fully-busy 4096-cycle window and
            # keeps the warm state as long as idle gaps stay < ~3.4us; enter
            # every later phase warm.
            dum_sb = persist.tile([128, 512], BF16)
            nc.vector.memset(dum_sb[:, :], 0.0)
            with tc.tile_pool(name="wu", bufs=2, space="PSUM") as wu:
                for i in range(24):
                    ps = wu.tile([128, 512], F32, tag="wu", name=f"wu{i}")
                    nc.tensor.matmul(out=ps, lhsT=dum_sb[:, 0:128],
                                     rhs=dum_sb[:, :], start=True, stop=True)

            # ---- fused projections + attention ----
            # Software-pipelined per head: exp(mt) -> S^T(mt+1) -> PV(mt), so
            # in PE program order S^T(mt+1) precedes PV(mt) and ACT's wait on
            # "S^T(mt+1) done" (monotonic PE tick) never transitively waits
            # on PV.  st bufs=3 (6 banks) + one PV accumulator (2 banks).
            # Projection tiles share the st slots and fill PE spare capacity.
            with tc.tile_pool(name="st", bufs=3, space="PSUM") as st_pool, \
                 tc.tile_pool(name="pv", bufs=1, space="PSUM") as pv_pool, \
                 tc.tile_pool(name="pb", bufs=8) as pb_pool, \
                 tc.tile_pool(name="nrm", bufs=4) as nrm_pool:

                def q_proj(ot):
                    ps = st_pool.tile([128, NL], F32, tag="st", name=f"qp{ot}")
                    for i in range(2):
                        for ct in range(2):
                            nc.tensor.matmul(
                                out=ps[:, i * 512:(i + 1) * 512],
                                lhsT=wq_sb[:, ct, ot * 128:(ot + 1) * 128],
                                rhs=xl_sb[:, ct, i * 512:(i + 1) * 512],
                                start=(ct == 0), stop=(ct == 1))
                    nc.vector.tensor_copy(out=q_sb[:, ot, :], in_=ps)

                def k_proj(ot, i):
                    ps = st_pool.tile([128, 512], F32, tag="st",
                                      name=f"kp{ot}_{i}")
                    for ct in range(2):
                        nc.tensor.matmul(
                            out=ps,
                            lhsT=wk_sb[:, ct, ot * 128:(ot + 1) * 128],
                            rhs=xf_sb[:, ct, i * 512:(i + 1) * 512],
                            start=(ct == 0), stop=(ct == 1))
                    nc.vector.tensor_copy(
                        out=k_sb[:, ot, i * 512:(i + 1) * 512], in_=ps)

                def vt_proj(mt):
                    ps = st_pool.tile([128, 260], F32, tag="st",
                                      name=f"vp{mt}")
                    for ct in range(2):
                        nc.tensor.matmul(
                            out=ps,
                            lhsT=xf_sb[:, ct, mt * 128:(mt + 1) * 128],
                            rhs=wva_sb[:, ct, :],
                            start=(ct == 0), stop=(ct == 1))
                    nc.vector.tensor_copy(
                        out=vt_sb[:, mt, :, 0:64],
                        in_=ps[:, 0:256].rearrange("p (h d) -> p h d", h=H))
                    nc.vector.tensor_copy(out=r_sb[:, mt, :], in_=ps[:, 256:260])

                def norm_msg(h, pv):
                    # evacuate PSUM accumulator right away (frees the pv slot
                    # for the next head); the rest runs off the critical path
                    pair, hh = h // 2, h % 2
                    rows = slice(hh * 64, (hh + 1) * 64)
                    pvc = nrm_pool.tile([65, NL], F32, tag="pvc",
                                        name=f"pvc{h}")
                    nc.vector.tensor_copy(out=pvc, in_=pv)
                    rcp = nrm_pool.tile([1, NL], F32, tag="rcp",
                                        name=f"rcp{h}")
                    nc.vector.reciprocal(out=rcp, in_=pvc[64:65, :])
                    rcp_d = dram.tile([1, NL], F32, tag="rcp_d",
                                      name=f"rcpd{h}")
                    nc.sync.dma_start(out=rcp_d, in_=rcp)
                    rb = nrm_pool.tile([64, NL], F32, tag="rb", name=f"rb{h}")
                    bc = bass.AP(tensor=rcp_d.tensor, offset=rcp_d.offset,
                                 ap=[[0, 64]] + list(rcp_d.ap[1:]))
                    nc.sync.dma_start(out=rb, in_=bc)
                    nc.vector.tensor_mul(
                        out=msg_sb[rows, pair, :], in0=pvc[0:64, :], in1=rb)

                def st_mm(h, mt):
                    pair, hh = h // 2, h % 2
                    rows = slice(hh * 64, (hh + 1) * 64)
                    ps = st_pool.tile([128, NL], F32, tag="st",
                                      name=f"s{h}_{mt}")
                    for i in range(2):
                        nc.tensor.matmul(
                            out=ps[:, i * 512:(i + 1) * 512],
                            lhsT=k_sb[rows, pair, mt * 128:(mt + 1) * 128],
                            rhs=q_sb[rows, pair, i * 512:(i + 1) * 512],
                            start=True, stop=True)
                    return ps

                q_proj(0)
                for j in range(3):
                    k_proj(0, j)
                for mtp in range(3):
                    vt_proj(mtp)

                pvs = {}
                for h in range(H):
                    pair = h // 2
                    pv = pv_pool.tile([65, NL], F32, tag="pv", name=f"pv{h}")
                    pvs[h] = pv
                    sts = st_mm(h, 0)
                    for mt in range(MT):
                        h_ = 2 * pair + (h % 2)
                        pt = pb_pool.tile([128, NL], BF16, tag="pt",
                                          name=f"p{h}_{mt}")
                        nc.scalar.activation(
                            out=pt, in_=sts, func=AF.Exp,
                            bias=r_sb[:, mt, h:h + 1], scale=0.125)
                        # projection ride-alongs (keep them ahead of use)
                        if h == 0:
                            if mt % 4 == 0 and 3 + mt // 4 < 16:
                                k_proj(0, 3 + mt // 4)
                            if mt + 3 < MT:
                                vt_proj(mt + 3)
                        elif h == 1:
                            if mt == 0:
                                q_proj(1)
                            if mt % 4 == 2 and mt // 4 < 16:
                                k_proj(1, mt // 4)
                        if mt + 1 < MT:
                            sts = st_mm(h, mt + 1)
                        for i in range(2):
                            nc.tensor.matmul(
                                out=pv[:, i * 512:(i + 1) * 512],
                                lhsT=vt_sb[:, mt, h, :],
                                rhs=pt[:, i * 512:(i + 1) * 512],
                                start=(mt == 0), stop=(mt == MT - 1))
                    norm_msg(h, pv)

            # ---- y = W1a@x + W1b@msg ; per-core stats ----
            with tc.tile_pool(name="yp", bufs=4, space="PSUM") as yp, \
                 tc.tile_pool(name="stat", bufs=1) as stp:
                # bridge the attention->y boundary (norm chain ~8us)
                for i in range(40):
                    ps = yp.tile([128, 512], F32, tag="yp", name=f"wy{i}")
                    nc.tensor.matmul(out=ps, lhsT=dum_sb[:, 0:128],
                                     rhs=dum_sb[:, :], start=True, stop=True)
                st_t = stp.tile([128, 4, 2, 6], F32)
                mv_t = stp.tile([128, 4, 2], F32)
                ar_in = stp.tile([128, 4, 2], F32)
                tmp_m2 = stp.tile([128, 1], F32)
                for ot in range(4):
                    for i in range(2):
                        ps = yp.tile([128, 512], F32, tag="yp")
                        for ct in range(2):
                            nc.tensor.matmul(
                                out=ps,
                                lhsT=w1a_sb[:, ct, ot * 128:(ot + 1) * 128],
                                rhs=xl_sb[:, ct, i * 512:(i + 1) * 512],
                                start=(ct == 0), stop=False)
                        for ct in range(2):
                            nc.tensor.matmul(
                                out=ps,
                                lhsT=w1b_sb[:, ct, ot * 128:(ot + 1) * 128],
                                rhs=msg_sb[:, ct, i * 512:(i + 1) * 512],
                                start=False, stop=(ct == 1))
                        nc.vector.bn_stats(out=st_t[:, ot, i, :], in_=ps)
                        nc.scalar.copy(
                            out=y_sb[:, ot, i * 512:(i + 1) * 512], in_=ps)
                    nc.vector.bn_aggr(out=mv_t[:, ot, :], in_=st_t[:, ot, :, :])
                    # (mean, E[y^2]) for cross-core combination
                    nc.vector.tensor_mul(out=tmp_m2, in0=mv_t[:, ot, 0:1],
                                         in1=mv_t[:, ot, 0:1])
                    nc.vector.tensor_add(out=ar_in[:, ot, 1:2],
                                         in0=mv_t[:, ot, 1:2], in1=tmp_m2)
                    nc.vector.tensor_copy(out=ar_in[:, ot, 0:1],
                                          in_=mv_t[:, ot, 0:1])

                # ---- AllReduce of [128,4,2] stats ----
                ar_i_d = dram.tile([128, 4, 2], F32, tag="ar_i")
                ar_o_d = dram.tile([128, 4, 2], F32, tag="ar_o")
                nc.sync.dma_start(out=ar_i_d, in_=ar_in)
                nc.gpsimd.collective_compute(
                    "AllReduce", ALU.add,
                    replica_groups=[list(range(NCORES))],
                    ins=[ar_i_d.opt()], outs=[ar_o_d.opt()])
                ar_out = stp.tile([128, 4, 2], F32)
                nc.sync.dma_start(out=ar_out, in_=ar_o_d)

                # stats finalize: rs = 1/sqrt(var+eps), nm = -mu*rs
                mu_t = stp.tile([128, 4], F32)
                ey_t = stp.tile([128, 4], F32)
                rs_t = stp.tile([128, 4], F32)
                nm_t = stp.tile([128, 4], F32)
                nc.vector.tensor_scalar_mul(out=mu_t, in0=ar_out[:, :, 0],
                                            scalar1=1.0 / NCORES)
                nc.vector.tensor_scalar_mul(out=ey_t, in0=ar_out[:, :, 1],
                                            scalar1=1.0 / NCORES)
                nc.vector.tensor_mul(out=rs_t, in0=mu_t, in1=mu_t)
                nc.vector.tensor_sub(out=ey_t, in0=ey_t, in1=rs_t)  # var
                nc.vector.tensor_scalar_add(out=ey_t, in0=ey_t, scalar1=EPS)
                nc.scalar.activation(out=rs_t, in_=ey_t, func=AF.Sqrt)
                nc.vector.reciprocal(out=rs_t, in_=rs_t)
                nc.vector.tensor_mul(out=nm_t, in0=mu_t, in1=rs_t)
                nc.vector.tensor_scalar_mul(out=nm_t, in0=nm_t, scalar1=-1.0)

                # ---- yn = relu(y*rs + nm) ----
                for ot in range(4):
                    for i in range(2):
                        sl = slice(i * 512, (i + 1) * 512)
                        nc.vector.tensor_scalar(
                            out=yn_sb[:, ot, sl], in0=y_sb[:, ot, sl],
                            scalar1=rs_t[:, ot:ot + 1],
                            scalar2=nm_t[:, ot:ot + 1],
                            op0=ALU.mult, op1=ALU.add)
                        nc.vector.tensor_scalar_max(
                            out=yn_sb[:, ot, sl], in0=yn_sb[:, ot, sl],
                            scalar1=0.0)

                # ---- delta = W2 @ yn -> DRAM ----
                out_sb = stp.tile([128, 2, NL], F32)
                for ot in range(2):
                    for i in range(2):
                        ps = yp.tile([128, 512], F32, tag="yp")
                        for ct in range(4):
                            nc.tensor.matmul(
                                out=ps,
                                lhsT=w2_sb[:, ct, ot * 128:(ot + 1) * 128],
                                rhs=yn_sb[:, ct, i * 512:(i + 1) * 512],
                                start=(ct == 0), stop=(ct == 3))
                        sl = slice(i * 512, (i + 1) * 512)
                        if i == 0:
                            nc.scalar.copy(out=out_sb[:, ot, sl], in_=ps)
                        else:
                            nc.vector.tensor_copy(out=out_sb[:, ot, sl], in_=ps)
                        nc.sync.dma_start(
                            out=delta_d[ot, :, sl], in_=out_sb[:, ot, sl])

    nc.compile()
    return nc


_NC_CACHE = {}


def _get_nc():
    if "nc" not in _NC_CACHE:
        _NC_CACHE["nc"] = build_kernel()
    return _NC_CACHE["nc"]


def _prep(a, shape):
    return np.ascontiguousarray(
        np.asarray(a, np.float32).astype(BF16_NP).reshape(shape))


def kernel(descs, Wq, bq, Wk, bk, Wv, bv, Wm, bm, W1, b1, W2, b2):
    descs = np.asarray(descs, np.float32)
    Wq, bq = np.asarray(Wq, np.float32), np.asarray(bq, np.float32)
    Wk, bk = np.asarray(Wk, np.float32), np.asarray(bk, np.float32)
    Wv, bv = np.asarray(Wv, np.float32), np.asarray(bv, np.float32)
    Wm, bm = np.asarray(Wm, np.float32), np.asarray(bm, np.float32)
    W1, b1 = np.asarray(W1, np.float32), np.asarray(b1, np.float32)
    W2, b2 = np.asarray(W2, np.float32), np.asarray(b2, np.float32)

    x = descs[0].T  # [C, N] fp32
    perm = np.array([d * H + h for h in range(H) for d in range(DIM)])
    Wqp, Wkp, Wvp, bqp = Wq[perm], Wk[perm], Wv[perm], bq[perm]
    # wva: columns 0..255 = Wv'^T, 256+h = Wk'_h^T bq'_h / 8, pad to 260
    wva = np.zeros((C, 260), np.float32)
    wva[:, :C] = Wvp.T
    for h in range(H):
        sl = slice(h * DIM, (h + 1) * DIM)
        wva[:, C + h] = (Wkp[sl].T @ bqp[sl]) / np.sqrt(DIM)
    W1a = W1[:, :C]
    W1b = W1[:, C:] @ Wm[:, perm]

    xf = _prep(x, (2, 128, N))
    wq = _prep(Wqp.T, (2, 128, C))
    wk = _prep(Wkp.T, (2, 128, C))
    wva_b = _prep(wva, (2, 128, 260))
    w1a = _prep(W1a.T, (2, 128, 2 * C))
    w1b = _prep(W1b.T, (2, 128, 2 * C))
    w2 = _prep(W2.T, (4, 128, C))

    in_maps = []
    for i in range(NCORES):
        xl = _prep(x[:, i * NL:(i + 1) * NL], (2, 128, NL))
        in_maps.append(dict(xl=xl, xf=xf, wq=wq, wk=wk, wva=wva_b,
                            w1a=w1a, w1b=w1b, w2=w2))

    nc = _get_nc()
    _NC_CACHE["in_maps"] = in_maps
    res = run_bass_kernel_spmd(nc, in_maps, core_ids=list(range(NCORES)))
    delta = np.concatenate(
        [r["delta"].reshape(C, NL) for r in res.results], axis=1)  # [C, N]
    out = descs[0] + delta.T + b2[None, :]
    return out[None].astype(np.float32)


# revision 21
# speedup vs baseline: 1.2434x; 1.0156x over previous
"""Trainium2 Bass kernel for the SuperGlue-style cross-attention block.

Math (validated in fp32 to ~2e-7 vs the jax reference):
  x = descs.T  [C=256, N=8192]
  q/k/v head-permuted so each head owns a contiguous 64-row block.
  Per head: S^T[m,n] = k_h[:,m].q_h[:,n];  P^T = exp(S^T/8 + r_h[m])  (no max
  subtraction needed: |S/8| < ~1);  r_h folds the bq bias exactly.
  PV with a ones-column appended to v^T gives unnormalized msg + denominator
  in one accumulated matmul; normalization happens on the PV output.
  Wm is folded into W1 host-side (W1b = W1[:,256:]@Wm).  bk/bv/bm/b1 are
  provably no-ops (softmax shift invariance / InstanceNorm mean removal);
  b2 and the residual are applied host-side.
  InstanceNorm statistics are combined across the 8 query-shard cores with a
  tiny [128,4,2] AllReduce.

Sharding: query axis N split 8 ways (sequence parallel); k/v computed from the
replicated full x on every core; params replicated.
"""

import numpy as np
import ml_dtypes

import concourse.bass as bass
import concourse.tile as tile
from concourse import bacc
import concourse.mybir as mybir
from concourse.bass_utils import run_bass_kernel_spmd

NCORES = 8
C = 256
N = 8192
H = 4
DIM = 64
NL = N // NCORES  # 1024 queries per core
EPS = 1e-5

F32 = mybir.dt.float32
BF16 = mybir.dt.bfloat16
AF = mybir.ActivationFunctionType
ALU = mybir.AluOpType
BF16_NP = ml_dtypes.bfloat16


def build_kernel(with_r=True):
    nc = bacc.Bacc("TRN2", target_bir_lowering=False, debug=False,
                   num_devices=NCORES)

    # ---- DRAM I/O (all weight marshalling is done host-side) ----
    xl_d = nc.dram_tensor("xl", [2, 128, NL], BF16, kind="ExternalInput")
    xf_d = nc.dram_tensor("xf", [2, 128, N], BF16, kind="ExternalInput")
    wq_d = nc.dram_tensor("wq", [2, 128, C], BF16, kind="ExternalInput")
    wk_d = nc.dram_tensor("wk", [2, 128, C], BF16, kind="ExternalInput")
    wva_d = nc.dram_tensor("wva", [2, 128, 260], BF16, kind="ExternalInput")
    w1a_d = nc.dram_tensor("w1a", [2, 128, 2 * C], BF16, kind="ExternalInput")
    w1b_d = nc.dram_tensor("w1b", [2, 128, 2 * C], BF16, kind="ExternalInput")
    w2_d = nc.dram_tensor("w2", [4, 128, C], BF16, kind="ExternalInput")
    delta_d = nc.dram_tensor("delta", [2, 128, NL], F32, kind="ExternalOutput")

    MT = N // 128  # 64 key tiles

    with tile.TileContext(nc) as tc:
        with tc.tile_pool(name="persist", bufs=1) as persist, \
             tc.tile_pool(name="dram", bufs=1, space="DRAM") as dram:

            # ---- persistent SBUF tensors ----
            xl_sb = persist.tile([128, 2, NL], BF16)
            xf_sb = persist.tile([128, 2, N], BF16)
            wq_sb = persist.tile([128, 2, C], BF16)
            wk_sb = persist.tile([128, 2, C], BF16)
            wva_sb = persist.tile([128, 2, 260], BF16)
            w1a_sb = persist.tile([128, 2, 2 * C], BF16)
            w1b_sb = persist.tile([128, 2, 2 * C], BF16)
            w2_sb = persist.tile([128, 4, C], BF16)
            q_sb = persist.tile([128, 2, NL], BF16)
            k_sb = persist.tile([128, 2, N], BF16)
            vt_sb = persist.tile([128, MT, H, 65], BF16)
            r_sb = persist.tile([128, MT, H], F32)
            msg_sb = persist.tile([128, 2, NL], BF16)
            yn_sb = persist.tile([128, 4, NL], BF16)

            for ct in range(2):
                nc.sync.dma_start(out=xl_sb[:, ct, :], in_=xl_d[ct])
                nc.sync.dma_start(out=wq_sb[:, ct, :], in_=wq_d[ct])
                nc.sync.dma_start(out=wk_sb[:, ct, :], in_=wk_d[ct])
                nc.sync.dma_start(out=wva_sb[:, ct, :], in_=wva_d[ct])
            for ch in range(8):  # chunk-major so early k tiles unblock fast
                s = slice(ch * (N // 8), (ch + 1) * (N // 8))
                for ct in range(2):
                    nc.sync.dma_start(out=xf_sb[:, ct, s], in_=xf_d[ct, :, s])
            for ct in range(2):
                nc.sync.dma_start(out=w1a_sb[:, ct, :], in_=w1a_d[ct])
                nc.sync.dma_start(out=w1b_sb[:, ct, :], in_=w1b_d[ct])
            for ct in range(4):
                nc.sync.dma_start(out=w2_sb[:, ct, :], in_=w2_d[ct])

            # ones column of v~^T
            nc.vector.memset(vt_sb[:, :, :, 64:65], 1.0)

            # ---- PE warmup: ~7us of dummy back-to-back matmuls ----
            # HAM only unthrottles after a fully-busy 4096-cycle window and
            # keeps the warm state as long as idle gaps stay < ~3.4us; enter
            # every later phase warm.
            dum_sb = persist.tile([128, 512], BF16)
            nc.vector.memset(dum_sb[:, :], 0.0)
            with tc.tile_pool(name="wu", bufs=2, space="PSUM") as wu:
                for i in range(16):
                    ps = wu.tile([128, 512], F32, tag="wu", name=f"wu{i}")
                    nc.tensor.matmul(out=ps, lhsT=dum_sb[:, 0:128],
                                     rhs=dum_sb[:, :], start=True, stop=True)

            # ---- fused projections + attention ----
            # Software-pipelined per head: exp(mt) -> S^T(mt+1) -> PV(mt), so
            # in PE program order S^T(mt+1) precedes PV(mt) and ACT's wait on
            # "S^T(mt+1) done" (monotonic PE tick) never transitively waits
            # on PV.  st bufs=3 (6 banks) + one PV accumulator (2 banks).
            # Projection tiles share the st slots and fill PE spare capacity.
            with tc.tile_pool(name="st", bufs=3, space="PSUM") as st_pool, \
                 tc.tile_pool(name="pv", bufs=1, space="PSUM") as pv_pool, \
                 tc.tile_pool(name="pb", bufs=8) as pb_pool, \
                 tc.tile_pool(name="nrm", bufs=4) as nrm_pool:

                def q_proj(ot):
                    ps = st_pool.tile([128, NL], F32, tag="st", name=f"qp{ot}")
                    for i in range(2):
                        for ct in range(2):
                            nc.tensor.matmul(
                                out=ps[:, i * 512:(i + 1) * 512],
                                lhsT=wq_sb[:, ct, ot * 128:(ot + 1) * 128],
                                rhs=xl_sb[:, ct, i * 512:(i + 1) * 512],
                                start=(ct == 0), stop=(ct == 1))
                    nc.vector.tensor_copy(out=q_sb[:, ot, :], in_=ps)

                def k_proj(ot, i):
                    ps = st_pool.tile([128, 512], F32, tag="st",
                                      name=f"kp{ot}_{i}")
                    for ct in range(2):
                        nc.tensor.matmul(
                            out=ps,
                            lhsT=wk_sb[:, ct, ot * 128:(ot + 1) * 128],
                            rhs=xf_sb[:, ct, i * 512:(i + 1) * 512],
                            start=(ct == 0), stop=(ct == 1))
                    nc.vector.tensor_copy(
                        out=k_sb[:, ot, i * 512:(i + 1) * 512], in_=ps)

                def vt_proj(mt):
                    ps = st_pool.tile([128, 260], F32, tag="st",
                                      name=f"vp{mt}")
                    for ct in range(2):
                        nc.tensor.matmul(
                            out=ps,
                            lhsT=xf_sb[:, ct, mt * 128:(mt + 1) * 128],
                            rhs=wva_sb[:, ct, :],
                            start=(ct == 0), stop=(ct == 1))
                    nc.vector.tensor_copy(
                        out=vt_sb[:, mt, :, 0:64],
                        in_=ps[:, 0:256].rearrange("p (h d) -> p h d", h=H))
                    if with_r:
                        nc.vector.tensor_copy(out=r_sb[:, mt, :],
                                              in_=ps[:, 256:260])

                def norm_msg(h, pv):
                    # evacuate PSUM accumulator right away (frees the pv slot
                    # for the next head); the rest runs off the critical path
                    pair, hh = h // 2, h % 2
                    rows = slice(hh * 64, (hh + 1) * 64)
                    pvc = nrm_pool.tile([65, NL], F32, tag="pvc",
                                        name=f"pvc{h}")
                    nc.vector.tensor_copy(out=pvc, in_=pv)
                    rcp = nrm_pool.tile([1, NL], F32, tag="rcp",
                                        name=f"rcp{h}")
                    nc.vector.reciprocal(out=rcp, in_=pvc[64:65, :])
                    rcp_d = dram.tile([1, NL], F32, tag="rcp_d",
                                      name=f"rcpd{h}")
                    nc.sync.dma_start(out=rcp_d, in_=rcp)
                    rb = nrm_pool.tile([64, NL], F32, tag="rb", name=f"rb{h}")
                    bc = bass.AP(tensor=rcp_d.tensor, offset=rcp_d.offset,
                                 ap=[[0, 64]] + list(rcp_d.ap[1:]))
                    nc.sync.dma_start(out=rb, in_=bc)
                    nc.vector.tensor_mul(
                        out=msg_sb[rows, pair, :], in0=pvc[0:64, :], in1=rb)

                def st_mm(h, mt):
                    pair, hh = h // 2, h % 2
                    rows = slice(hh * 64, (hh + 1) * 64)
                    ps = st_pool.tile([128, NL], F32, tag="st",
                                      name=f"s{h}_{mt}")
                    for i in range(2):
                        nc.tensor.matmul(
                            out=ps[:, i * 512:(i + 1) * 512],
                            lhsT=k_sb[rows, pair, mt * 128:(mt + 1) * 128],
                            rhs=q_sb[rows, pair, i * 512:(i + 1) * 512],
                            start=True, stop=True)
                    return ps

                q_proj(0)
                for j in range(3):
                    k_proj(0, j)
                for mtp in range(3):
                    vt_proj(mtp)

                pvs = {}
                for h in range(H):
                    pair = h // 2
                    pv = pv_pool.tile([65, NL], F32, tag="pv", name=f"pv{h}")
                    pvs[h] = pv
                    sts = st_mm(h, 0)
                    for mt in range(MT):
                        h_ = 2 * pair + (h % 2)
                        pt = pb_pool.tile([128, NL], BF16, tag="pt",
                                          name=f"p{h}_{mt}")
                        if with_r:
                            nc.scalar.activation(
                                out=pt, in_=sts, func=AF.Exp,
                                bias=r_sb[:, mt, h:h + 1], scale=0.125)
                        else:
                            nc.scalar.activation(
                                out=pt, in_=sts, func=AF.Exp, scale=0.125)
                        # projection ride-alongs (keep them ahead of use)
                        if h == 0:
                            if mt % 4 == 0 and 3 + mt // 4 < 16:
                                k_proj(0, 3 + mt // 4)
                            if mt + 3 < MT:
                                vt_proj(mt + 3)
                        elif h == 1:
                            if mt == 0:
                                q_proj(1)
                            if mt % 4 == 2 and mt // 4 < 16:
                                k_proj(1, mt // 4)
                        if mt + 1 < MT:
                            sts = st_mm(h, mt + 1)
                        for i in range(2):
                            nc.tensor.matmul(
                                out=pv[:, i * 512:(i + 1) * 512],
                                lhsT=vt_sb[:, mt, h, :],
                                rhs=pt[:, i * 512:(i + 1) * 512],
                                start=(mt == 0), stop=(mt == MT - 1))
                    norm_msg(h, pv)

            # ---- y = W1a@x + W1b@msg ; stats; yn fused on ACT ----
            # y stays resident in PSUM (8 tiles = 8 banks) across the
            # AllReduce; yn = Relu(y*rs + nm) is then a single ACT op per
            # tile straight out of PSUM.
            with tc.tile_pool(name="yp", bufs=8, space="PSUM") as yp, \
                 tc.tile_pool(name="stat", bufs=1) as stp:
                # bridge the attention->y boundary (norm chain ~8us)
                for i in range(24):
                    ps = yp.tile([128, 512], F32, tag="yp", name=f"wy{i}",
                                 bufs=8)
                    nc.tensor.matmul(out=ps, lhsT=dum_sb[:, 0:128],
                                     rhs=dum_sb[:, :], start=True, stop=True)
                st_t = stp.tile([128, 4, 2, 6], F32)
                mv_t = stp.tile([128, 4, 2], F32)
                ar_in = stp.tile([128, 4, 2], F32)
                tmp_m2 = stp.tile([128, 1], F32)
                y_ps = {}
                for ot in range(4):
                    for i in range(2):
                        ps = yp.tile([128, 512], F32, tag="yp",
                                     name=f"y{ot}_{i}", bufs=8)
                        y_ps[(ot, i)] = ps
                        for ct in range(2):
                            nc.tensor.matmul(
                                out=ps,
                                lhsT=w1a_sb[:, ct, ot * 128:(ot + 1) * 128],
                                rhs=xl_sb[:, ct, i * 512:(i + 1) * 512],
                                start=(ct == 0), stop=False)
                        for ct in range(2):
                            nc.tensor.matmul(
                                out=ps,
                                lhsT=w1b_sb[:, ct, ot * 128:(ot + 1) * 128],
                                rhs=msg_sb[:, ct, i * 512:(i + 1) * 512],
                                start=False, stop=(ct == 1))
                        nc.vector.bn_stats(out=st_t[:, ot, i, :], in_=ps)
                    nc.vector.bn_aggr(out=mv_t[:, ot, :], in_=st_t[:, ot, :, :])
                    # (mean, E[y^2]) for cross-core combination
                    nc.vector.tensor_mul(out=tmp_m2, in0=mv_t[:, ot, 0:1],
                                         in1=mv_t[:, ot, 0:1])
                    nc.vector.tensor_add(out=ar_in[:, ot, 1:2],
                                         in0=mv_t[:, ot, 1:2], in1=tmp_m2)
                    nc.vector.tensor_copy(out=ar_in[:, ot, 0:1],
                                          in_=mv_t[:, ot, 0:1])

                # ---- AllReduce of [128,4,2] stats ----
                ar_i_d = dram.tile([128, 4, 2], F32, tag="ar_i")
                ar_o_d = dram.tile([128, 4, 2], F32, tag="ar_o")
                nc.sync.dma_start(out=ar_i_d, in_=ar_in)
                nc.gpsimd.collective_compute(
                    "AllReduce", ALU.add,
                    replica_groups=[list(range(NCORES))],
                    ins=[ar_i_d.opt()], outs=[ar_o_d.opt()])
                ar_out = stp.tile([128, 4, 2], F32)
                nc.sync.dma_start(out=ar_out, in_=ar_o_d)

                # stats finalize: rs = 1/sqrt(var+eps), nm = -mu*rs
                mu_t = stp.tile([128, 4], F32)
                ey_t = stp.tile([128, 4], F32)
                rs_t = stp.tile([128, 4], F32)
                nm_t = stp.tile([128, 4], F32)
                nc.vector.tensor_scalar_mul(out=mu_t, in0=ar_out[:, :, 0],
                                            scalar1=1.0 / NCORES)
                nc.vector.tensor_scalar_mul(out=ey_t, in0=ar_out[:, :, 1],
                                            scalar1=1.0 / NCORES)
                nc.vector.tensor_mul(out=rs_t, in0=mu_t, in1=mu_t)
                nc.vector.tensor_sub(out=ey_t, in0=ey_t, in1=rs_t)  # var
                nc.vector.tensor_scalar_add(out=ey_t, in0=ey_t, scalar1=EPS)
                nc.scalar.activation(out=rs_t, in_=ey_t, func=AF.Sqrt)
                nc.vector.reciprocal(out=rs_t, in_=rs_t)
                nc.vector.tensor_mul(out=nm_t, in0=mu_t, in1=rs_t)
                nc.vector.tensor_scalar_mul(out=nm_t, in0=nm_t, scalar1=-1.0)

                # ---- yn = relu(y*rs + nm), one ACT op per PSUM tile ----
                for ot in range(4):
                    for i in range(2):
                        sl = slice(i * 512, (i + 1) * 512)
                        nc.scalar.activation(
                            out=yn_sb[:, ot, sl], in_=y_ps[(ot, i)],
                            func=AF.Relu, bias=nm_t[:, ot:ot + 1],
                            scale=rs_t[:, ot:ot + 1])

                # ---- delta = W2 @ yn -> DRAM ----
                out_sb = stp.tile([128, 2, NL], F32)
                for ot in range(2):
                    for i in range(2):
                        ps = yp.tile([128, 512], F32, tag="yp")
                        for ct in range(4):
                            nc.tensor.matmul(
                                out=ps,
                                lhsT=w2_sb[:, ct, ot * 128:(ot + 1) * 128],
                                rhs=yn_sb[:, ct, i * 512:(i + 1) * 512],
                                start=(ct == 0), stop=(ct == 3))
                        sl = slice(i * 512, (i + 1) * 512)
                        if i == 0:
                            nc.scalar.copy(out=out_sb[:, ot, sl], in_=ps)
                        else:
                            nc.vector.tensor_copy(out=out_sb[:, ot, sl], in_=ps)
                        nc.sync.dma_start(
                            out=delta_d[ot, :, sl], in_=out_sb[:, ot, sl])

    nc.compile()
    return nc


_NC_CACHE = {}


def _get_nc(with_r=False):
    key = f"nc{int(with_r)}"
    if key not in _NC_CACHE:
        _NC_CACHE[key] = build_kernel(with_r=with_r)
    return _NC_CACHE[key]


def _prep(a, shape):
    return np.ascontiguousarray(
        np.asarray(a, np.float32).astype(BF16_NP).reshape(shape))


def kernel(descs, Wq, bq, Wk, bk, Wv, bv, Wm, bm, W1, b1, W2, b2):
    descs = np.asarray(descs, np.float32)
    Wq, bq = np.asarray(Wq, np.float32), np.asarray(bq, np.float32)
    Wk, bk = np.asarray(Wk, np.float32), np.asarray(bk, np.float32)
    Wv, bv = np.asarray(Wv, np.float32), np.asarray(bv, np.float32)
    Wm, bm = np.asarray(Wm, np.float32), np.asarray(bm, np.float32)
    W1, b1 = np.asarray(W1, np.float32), np.asarray(b1, np.float32)
    W2, b2 = np.asarray(W2, np.float32), np.asarray(b2, np.float32)

    x = descs[0].T  # [C, N] fp32
    perm = np.array([d * H + h for h in range(H) for d in range(DIM)])
    Wqp, Wkp, Wvp, bqp = Wq[perm], Wk[perm], Wv[perm], bq[perm]
    # wva: columns 0..255 = Wv'^T, 256+h = Wk'_h^T bq'_h / 8, pad to 260
    wva = np.zeros((C, 260), np.float32)
    wva[:, :C] = Wvp.T
    for h in range(H):
        sl = slice(h * DIM, (h + 1) * DIM)
        wva[:, C + h] = (Wkp[sl].T @ bqp[sl]) / np.sqrt(DIM)
    W1a = W1[:, :C]
    W1b = W1[:, C:] @ Wm[:, perm]

    xf = _prep(x, (2, 128, N))
    wq = _prep(Wqp.T, (2, 128, C))
    wk = _prep(Wkp.T, (2, 128, C))
    wva_b = _prep(wva, (2, 128, 260))
    w1a = _prep(W1a.T, (2, 128, 2 * C))
    w1b = _prep(W1b.T, (2, 128, 2 * C))
    w2 = _prep(W2.T, (4, 128, C))

    in_maps = []
    for i in range(NCORES):
        xl = _prep(x[:, i * NL:(i + 1) * NL], (2, 128, NL))
        in_maps.append(dict(xl=xl, xf=xf, wq=wq, wk=wk, wva=wva_b,
                            w1a=w1a, w1b=w1b, w2=w2))

    nc = _get_nc(with_r=bool(np.abs(bq).max() > 0))
    _NC_CACHE["in_maps"] = in_maps
    _NC_CACHE["nc"] = nc
    res = run_bass_kernel_spmd(nc, in_maps, core_ids=list(range(NCORES)))
    delta = np.concatenate(
        [r["delta"].reshape(C, NL) for r in res.results], axis=1)  # [C, N]
    out = descs[0] + delta.T + b2[None, :]
    return out[None].astype(np.float32)
